# revision 14
# baseline (speedup 1.0000x reference)
"""Tensor-parallel MiniGPT single-token decode step on 8 Trainium2 NeuronCores.

Sharding (per core i of 8):
  - attention: heads 2i, 2i+1 (head_dim 128 -> cols i*256:(i+1)*256 of E=2048);
    wq/wk/wv row-sharded, wo column-sharded, KV cache column-sharded by head.
  - MLP: w1 row-sharded (1024 rows/core), w2 column-sharded.
  - LM head: vocab-sharded (50257 padded to 8*6283=50264 rows).
  - Two 8KB AllReduces combine the wo- and w2- partial sums; logits are
    gathered on the host.

All weights are pre-laid-out on the host into [128, F] partition-major arrays
so every device DMA is one contiguous run per partition. All compute is fp32.

Matvec strategy: fp32 PE matmuls with a [128, 1] stationary run at only
~32 MACs/cycle, so the contraction is spread across three engines:
  - DVE / GPSIMD: acc[p, n] += W_tile[p, n] * x[k*128+p] via
    scalar_tensor_tensor, then one PE ones-vector matmul per 512-col chunk
    reduces across partitions (fp32 throughout).
  - PE: direct fp32 matmul chains (lhsT = x chunk) for a share of columns.
This keeps every engine's busy time below the DMA streaming time.
"""

import numpy as np

N_CORES = 8
E = 2048
HPC = 2  # heads per core
EPC = HPC * 128  # 256
T = 8192
VOCAB = 50257
VPC = 6283  # padded vocab rows per core (8 * 6283 = 50264)
SCALE = float(1.0 / np.sqrt(128.0))
EPS = 1e-5

_CACHE = {}
TRACE = False


def _build_nc():
    import concourse.bacc as bacc
    import concourse.mybir as mybir
    import concourse.tile as tile
    from concourse.masks import make_identity

    AF = mybir.ActivationFunctionType
    AX = mybir.AxisListType
    MUL = mybir.AluOpType.mult
    ADD = mybir.AluOpType.add
    dt = mybir.dt.float32

    nc = bacc.Bacc(
        "TRN2", target_bir_lowering=False, debug=False, num_devices=N_CORES
    )

    xe_wte = nc.declare_dram_parameter("xe_wte", [128, 16], dt, isOutput=False)
    xe_wpe = nc.declare_dram_parameter("xe_wpe", [128, 16], dt, isOutput=False)
    wqkv_r = nc.declare_dram_parameter("wqkv_r", [128, 16 * 768], dt, isOutput=False)
    keys_r = nc.declare_dram_parameter("keys_r", [128, 2 * 8192], dt, isOutput=False)
    vals_r = nc.declare_dram_parameter("vals_r", [128, 64 * 256], dt, isOutput=False)
    wo_r = nc.declare_dram_parameter("wo_r", [128, 2 * 2048], dt, isOutput=False)
    w1_r = nc.declare_dram_parameter("w1_r", [128, 16 * 1024], dt, isOutput=False)
    w2_r = nc.declare_dram_parameter("w2_r", [128, 8 * 2048], dt, isOutput=False)
    lm_r = nc.declare_dram_parameter("lm_r", [128, 16 * VPC], dt, isOutput=False)
    logits_out = nc.declare_dram_parameter("logits", [1, VPC], dt, isOutput=True)

    with tile.TileContext(nc) as tc:
        with (
            tc.tile_pool(name="const", bufs=1) as const,
            tc.tile_pool(name="small", bufs=1) as small,
            tc.tile_pool(name="stage", bufs=2) as stage,
            tc.tile_pool(name="ps", bufs=8, space="PSUM") as ps,
            tc.tile_pool(name="dram", bufs=1, space="DRAM") as dram,
            tc.tile_pool(name="stream", bufs=6) as stream,
            tc.tile_pool(name="acc", bufs=2) as accp,
        ):
            _snum = [0]

            def stile(label, width=4096):
                _snum[0] += 1
                return stream.tile(
                    [128, width], dt, tag="s", name=f"s{_snum[0]}_{label}"
                )

            ones_col = const.tile([128, 1], dt)
            nc.vector.memset(ones_col[:], 1.0)
            ones_row = const.tile([1, 128], dt)
            nc.vector.memset(ones_row[:], 1.0)
            ident = const.tile([128, 128], dt)
            make_identity(nc, ident[:])
            eps_c = const.tile([1, 1], dt)
            nc.vector.memset(eps_c[:], EPS)

            # Warm up the collectives path off the critical path: the first
            # collective in a NEFF pays a ~13us init cost.
            warm_in = dram.tile([1, 16], dt, tag="warm_in")
            warm_out = dram.tile([N_CORES, 16], dt, tag="warm_out")
            warm_sb = stage.tile([1, 16], dt, tag="warm", bufs=1)
            nc.vector.memset(warm_sb[:], 0.0)
            nc.scalar.dma_start(warm_in[:], warm_sb[:])
            nc.gpsimd.collective_compute(
                "AllGather",
                mybir.AluOpType.bypass,
                replica_groups=[list(range(N_CORES))],
                ins=[warm_in.opt()],
                outs=[warm_out.opt()],
            )
            warm_back = stage.tile([1, 16], dt, tag="warmb", bufs=1)
            nc.scalar.dma_start(warm_back[:], warm_out[0:1, :])

            def rms(xt, name, extra=None):
                """x * rsqrt(mean(x^2) + eps) for x in [128, 16] column layout.

                extra: optional [1, 16]-shaped nuisance row added with weight 0
                (keeps the warmup collective's output alive against DCE).
                """
                sq = small.tile([128, 16], dt, tag=f"sq_{name}")
                ssum = small.tile([128, 1], dt, tag=f"ss_{name}")
                nc.scalar.activation(sq[:], xt[:], AF.Square, accum_out=ssum[:])
                tot = ps.tile([1, 1], dt, tag="b")
                nc.tensor.matmul(tot[:], ssum[:], ones_col[:], start=True, stop=True)
                std = small.tile([1, 1], dt, tag=f"std_{name}")
                nc.scalar.activation(
                    std[:], tot[:], AF.Sqrt, bias=eps_c[:], scale=1.0 / float(E)
                )
                inv = small.tile([1, 1], dt, tag=f"inv_{name}")
                nc.vector.reciprocal(inv[:], std[:])
                invb_ps = ps.tile([128, 1], dt, tag="b")
                nc.tensor.matmul(
                    invb_ps[:], ones_row[:], inv[:], start=True, stop=True
                )
                invb = small.tile([128, 1], dt, tag=f"invb_{name}")
                nc.vector.tensor_copy(invb[:], invb_ps[:])
                xn = small.tile([128, 16], dt, tag=f"xn_{name}")
                nc.vector.tensor_scalar_mul(xn[:], xt[:], invb[:])
                return xn

            # ---- embedding + double rms ----
            xw = stage.tile([128, 16], dt, tag="xw")
            nc.scalar.dma_start(xw[:], xe_wte[:])
            xp = stage.tile([128, 16], dt, tag="xp")
            nc.scalar.dma_start(xp[:], xe_wpe[:])
            x0 = small.tile([128, 16], dt, tag="x0")
            nc.vector.tensor_add(x0[:], xw[:], xp[:])
            # keep the warmup-collective result alive: x0[0, :] += 0 * warm
            nc.vector.scalar_tensor_tensor(
                x0[0:1, :], warm_back[:], 0.0, x0[0:1, :], op0=MUL, op1=ADD
            )
            x1 = rms(x0, "n1")  # residual input
            x2 = rms(x1, "n2")

            # ---- qkv projection: [1, 768] row (q 0:256 | k 256:512 | v 512:768)
            acc_qkv = accp.tile([128, 768], dt, tag="acc", name="acc_qkv")
            for tt in range(4):
                wt = stile("wqkv", 3072)
                nc.sync.dma_start(wt[:], wqkv_r[:, tt * 3072 : (tt + 1) * 3072])
                for b in range(4):
                    k = tt * 4 + b
                    wsl = wt[:, b * 768 : (b + 1) * 768]
                    if k == 0:
                        nc.vector.tensor_scalar_mul(acc_qkv[:], wsl, x2[:, 0:1])
                    else:
                        nc.vector.scalar_tensor_tensor(
                            acc_qkv[:], wsl, x2[:, k : k + 1], acc_qkv[:],
                            op0=MUL, op1=ADD,
                        )
            ps_q = ps.tile([1, 512], dt, tag="b")
            nc.tensor.matmul(ps_q[:], ones_col[:], acc_qkv[:, 0:512], start=True, stop=True)
            ps_v = ps.tile([1, 256], dt, tag="b")
            nc.tensor.matmul(ps_v[:], ones_col[:], acc_qkv[:, 512:768], start=True, stop=True)
            qkv_row = small.tile([1, 768], dt, tag="qkv")
            nc.scalar.mul(qkv_row[:, 0:256], ps_q[:, 0:256], SCALE)
            nc.scalar.copy(qkv_row[:, 256:512], ps_q[:, 256:512])
            nc.scalar.copy(qkv_row[:, 512:768], ps_v[:])

            # ---- transpose q,k to column layout: qkT[:, 0:2]=q heads, 2:4=k heads
            st4 = stage.tile([4, 128], dt, tag="st4")
            nc.scalar.dma_start(st4[:], qkv_row[:, 0:512])
            qkT_ps = ps.tile([128, 4], dt, tag="b")
            nc.tensor.transpose(qkT_ps[:], st4[:], ident[0:4, 0:4])
            qkT = small.tile([128, 4], dt, tag="qkT")
            nc.vector.tensor_copy(qkT[:], qkT_ps[:])

            # ---- attention scores; exp applied post-transpose on [128, 16]
            # tiles. wT_h[p, c] = exp(att_h[c*128 + p]) (unnormalized).
            wTs = []
            esp = small.tile([128, 16], dt, tag="esp")  # per-partition exp sums
            for h in range(HPC):
                wTs.append(small.tile([128, 64], dt, tag=f"wT{h}", name=f"wT{h}"))
            for j2 in range(2):
                for h in range(HPC):
                    kt = stile("key")
                    nc.sync.dma_start(
                        kt[:],
                        keys_r[:, h * 8192 + j2 * 4096 : h * 8192 + (j2 + 1) * 4096],
                    )
                    for jj in range(2):
                        j = j2 * 2 + jj
                        att_row = small.tile(
                            [1, 2048], dt, tag="attrow", name=f"attrow{h}_{j}", bufs=2
                        )
                        for n in range(4):
                            pa = ps.tile([1, 512], dt, tag="b")
                            nc.tensor.matmul(
                                pa[:], qkT[:, h : h + 1],
                                kt[:, jj * 2048 + n * 512 : jj * 2048 + (n + 1) * 512],
                                start=True, stop=True,
                            )
                            if n % 2 == 0:
                                nc.vector.tensor_copy(
                                    att_row[:, n * 512 : (n + 1) * 512], pa[:]
                                )
                            else:
                                nc.scalar.copy(
                                    att_row[:, n * 512 : (n + 1) * 512], pa[:]
                                )
                        # reshape [1,2048] -> [16,128] -> transpose -> [128,16]
                        wst = stage.tile(
                            [16, 128], dt, tag="wst", name=f"wst{h}_{j}", bufs=4
                        )
                        nc.scalar.dma_start(wst[:], att_row[:])
                        wps = ps.tile([128, 16], dt, tag="b", name=f"wps{h}_{j}")
                        nc.tensor.transpose(wps[:], wst[:], ident[0:16, 0:16])
                        nc.scalar.activation(
                            wTs[h][:, j * 16 : (j + 1) * 16], wps[:], AF.Exp,
                            accum_out=esp[:, h * 4 + j : h * 4 + j + 1],
                        )

            # current-token score per head: exp(q_h . k_h) (SCALE folded into q)
            e_last = small.tile([1, 2], dt, tag="elast")
            for h in range(HPC):
                pal = ps.tile([1, 1], dt, tag="b")
                nc.tensor.matmul(
                    pal[:], qkT[:, h : h + 1], qkT[:, 2 + h : 3 + h],
                    start=True, stop=True,
                )
                nc.scalar.activation(e_last[:, h : h + 1], pal[:], AF.Exp)

            # softmax denominators: cross-partition sum of esp + e_last
            dps = ps.tile([1, 8], dt, tag="b")
            nc.tensor.matmul(dps[:], ones_col[:], esp[:, 0:8], start=True, stop=True)
            dtmp = small.tile([1, 2], dt, tag="dtmp")
            for h in range(HPC):
                nc.vector.reduce_sum(
                    dtmp[:, h : h + 1], dps[:, h * 4 : (h + 1) * 4], axis=AX.X
                )
            nc.vector.tensor_add(dtmp[:], dtmp[:], e_last[:])
            dinv = small.tile([1, 2], dt, tag="dinv")
            nc.vector.reciprocal(dinv[:], dtmp[:])

            # ---- PV: x_attn_h = sum_t w[t] * V[t, :] (unnormalized) on DVE
            acc_pv = [
                accp.tile([128, 128], dt, tag=f"accpv{h}", name=f"acc_pv{h}")
                for h in range(HPC)
            ]
            for tt in range(4):
                vt = stile("val")
                nc.sync.dma_start(vt[:], vals_r[:, tt * 4096 : (tt + 1) * 4096])
                for j in range(16):
                    c = tt * 16 + j
                    for h in range(HPC):
                        vsl = vt[:, j * 256 + h * 128 : j * 256 + (h + 1) * 128]
                        if c == 0:
                            nc.vector.tensor_scalar_mul(
                                acc_pv[h][:], vsl, wTs[h][:, 0:1]
                            )
                        else:
                            nc.vector.scalar_tensor_tensor(
                                acc_pv[h][:], vsl, wTs[h][:, c : c + 1], acc_pv[h][:],
                                op0=MUL, op1=ADD,
                            )
            pv_ps = []
            for h in range(HPC):
                p = ps.tile([1, 128], dt, tag="b", name=f"pv_ps{h}")
                nc.tensor.matmul(p[:], ones_col[:], acc_pv[h][:], start=True, stop=True)
                pv_ps.append(p)

            # combine with current-token value, then normalize by the softmax sum
            xa_row = small.tile([1, 256], dt, tag="xa")
            for h in range(HPC):
                sl = slice(h * 128, (h + 1) * 128)
                nc.vector.tensor_scalar_mul(
                    xa_row[:, sl],
                    qkv_row[:, 512 + h * 128 : 512 + (h + 1) * 128],
                    e_last[:, h : h + 1],
                )
                nc.vector.tensor_add(xa_row[:, sl], xa_row[:, sl], pv_ps[h][:])
                nc.vector.tensor_scalar_mul(xa_row[:, sl], xa_row[:, sl], dinv[:, h : h + 1])

            # ---- transpose x_attn to column layout [128, 2] ----
            st2 = stage.tile([2, 128], dt, tag="st2")
            nc.scalar.dma_start(st2[:], xa_row[:])
            xaT_ps = ps.tile([128, 2], dt, tag="b")
            nc.tensor.transpose(xaT_ps[:], st2[:], ident[0:2, 0:2])
            xaT = small.tile([128, 2], dt, tag="xaT")
            nc.vector.tensor_copy(xaT[:], xaT_ps[:])

            # ---- wo partial: [1, 2048]; DVE cols 0:1024, PE chains 1024:2048
            ar1_in = small.tile([1, 2048], dt, tag="arin", name="ar1_in")
            acc_wo = accp.tile([128, 1024], dt, tag="acc", name="acc_wo")
            wo_pe = [ps.tile([1, 512], dt, tag="b", name=f"wope{n}") for n in range(2)]
            wot = stile("wo")
            nc.sync.dma_start(wot[:], wo_r[:])
            for k in range(2):
                wsl = wot[:, k * 2048 : k * 2048 + 1024]
                if k == 0:
                    nc.vector.tensor_scalar_mul(acc_wo[:], wsl, xaT[:, 0:1])
                else:
                    nc.vector.scalar_tensor_tensor(
                        acc_wo[:], wsl, xaT[:, 1:2], acc_wo[:], op0=MUL, op1=ADD
                    )
                for n in range(2):
                    nc.tensor.matmul(
                        wo_pe[n][:], xaT[:, k : k + 1],
                        wot[:, k * 2048 + 1024 + n * 512 : k * 2048 + 1024 + (n + 1) * 512],
                        start=(k == 0), stop=(k == 1),
                    )
            for n in range(2):
                po = ps.tile([1, 512], dt, tag="b")
                nc.tensor.matmul(
                    po[:], ones_col[:], acc_wo[:, n * 512 : (n + 1) * 512],
                    start=True, stop=True,
                )
                nc.vector.tensor_copy(ar1_in[:, n * 512 : (n + 1) * 512], po[:])
            for n in range(2):
                nc.vector.tensor_copy(
                    ar1_in[:, 1024 + n * 512 : 1024 + (n + 1) * 512], wo_pe[n][:]
                )

            def all_reduce(row_sb, name):
                """Sum a [1, 2048] partial across cores; returns PSUM [128, 16].

                AllGather + on-core rank reduction: 16 K=8 matmuls against a
                ones vector sum the 8 gathered rows and transpose into the
                [128, 16] column layout.
                """
                in_d = dram.tile([1, 2048], dt, tag=f"{name}_in")
                out_d = dram.tile([N_CORES, 2048], dt, tag=f"{name}_out")
                nc.scalar.dma_start(in_d[:], row_sb[:])
                nc.gpsimd.collective_compute(
                    "AllGather",
                    mybir.AluOpType.bypass,
                    replica_groups=[list(range(N_CORES))],
                    ins=[in_d.opt()],
                    outs=[out_d.opt()],
                )
                ag_sb = stage.tile(
                    [N_CORES, 2048], dt, tag="ag", name=f"ag_{name}", bufs=1
                )
                nc.scalar.dma_start(ag_sb[:], out_d[:])
                x_ps = ps.tile([128, 16], dt, tag="b", name=f"xps_{name}")
                for c in range(16):
                    nc.tensor.matmul(
                        x_ps[:, c : c + 1],
                        ag_sb[:, c * 128 : (c + 1) * 128],
                        ones_col[0:N_CORES, :],
                        start=True, stop=True,
                    )
                return x_ps

            x3_ps = all_reduce(ar1_in, "ar1")
            x3 = small.tile([128, 16], dt, tag="x3")
            nc.vector.tensor_add(x3[:], x3_ps[:], x1[:])  # + residual

            # ---- MLP1: h = relu(w1 @ x4) on DVE ----
            x4 = rms(x3, "n3")
            acc_h1 = accp.tile([128, 1024], dt, tag="acc", name="acc_h1")
            for a in range(4):
                w1t = stile("w1")
                nc.sync.dma_start(w1t[:], w1_r[:, a * 4096 : (a + 1) * 4096])
                for b in range(4):
                    k = a * 4 + b
                    wsl = w1t[:, b * 1024 : (b + 1) * 1024]
                    if k == 0:
                        nc.vector.tensor_scalar_mul(acc_h1[:], wsl, x4[:, 0:1])
                    else:
                        nc.vector.scalar_tensor_tensor(
                            acc_h1[:], wsl, x4[:, k : k + 1], acc_h1[:],
                            op0=MUL, op1=ADD,
                        )
            h_row = small.tile([1, 1024], dt, tag="hrow")
            for n in range(2):
                phn = ps.tile([1, 512], dt, tag="b", name=f"ph{n}")
                nc.tensor.matmul(
                    phn[:], ones_col[:], acc_h1[:, n * 512 : (n + 1) * 512],
                    start=True, stop=True,
                )
                nc.scalar.activation(h_row[:, n * 512 : (n + 1) * 512], phn[:], AF.Relu)

            st8 = stage.tile([8, 128], dt, tag="st8")
            nc.scalar.dma_start(st8[:], h_row[:])
            hT_ps = ps.tile([128, 8], dt, tag="b")
            nc.tensor.transpose(hT_ps[:], st8[:], ident[0:8, 0:8])
            hT = small.tile([128, 8], dt, tag="hT")
            nc.vector.tensor_copy(hT[:], hT_ps[:])

            # ---- MLP2: DVE cols 0:1024 of each k-block, PE chains 1024:2048
            ar2_in = small.tile([1, 2048], dt, tag="arin", name="ar2_in")
            acc_m2 = accp.tile([128, 1024], dt, tag="acc", name="acc_m2")
            pm_pe = [ps.tile([1, 512], dt, tag="b", name=f"pmpe{n}") for n in range(2)]
            for a in range(4):
                w2t = stile("w2")
                nc.sync.dma_start(w2t[:], w2_r[:, a * 4096 : (a + 1) * 4096])
                for b in range(2):
                    k = a * 2 + b
                    wsl = w2t[:, b * 2048 : b * 2048 + 1024]
                    if k == 0:
                        nc.vector.tensor_scalar_mul(acc_m2[:], wsl, hT[:, 0:1])
                    else:
                        nc.vector.scalar_tensor_tensor(
                            acc_m2[:], wsl, hT[:, k : k + 1], acc_m2[:],
                            op0=MUL, op1=ADD,
                        )
                    for n in range(2):
                        nc.tensor.matmul(
                            pm_pe[n][:], hT[:, k : k + 1],
                            w2t[:, b * 2048 + 1024 + n * 512 : b * 2048 + 1024 + (n + 1) * 512],
                            start=(k == 0), stop=(k == 7),
                        )
            for n in range(2):
                pm = ps.tile([1, 512], dt, tag="b", name=f"pm{n}")
                nc.tensor.matmul(
                    pm[:], ones_col[:], acc_m2[:, n * 512 : (n + 1) * 512],
                    start=True, stop=True,
                )
                nc.vector.tensor_copy(ar2_in[:, n * 512 : (n + 1) * 512], pm[:])
            for n in range(2):
                nc.vector.tensor_copy(
                    ar2_in[:, 1024 + n * 512 : 1024 + (n + 1) * 512], pm_pe[n][:]
                )

            x5_ps = all_reduce(ar2_in, "ar2")
            x5 = small.tile([128, 16], dt, tag="x5")
            nc.vector.tensor_add(x5[:], x5_ps[:], x3[:])  # + residual (x3)

            # ---- LM head over the vocab shard, two passes of 512-col chunks.
            # One [128, width] tile per contraction block; DVE accumulates cols
            # 0:1536, PE runs direct fp32 matmul chains for the rest.
            lrow = small.tile([1, 512], dt, tag="lrow", bufs=3)
            passes = [(0, 3584), (3584, VPC)]
            DVE_W = 1536
            for lo, hi in passes:
                width = hi - lo
                acc_lm = accp.tile([128, DVE_W], dt, tag="acclm", name=f"acc_lm{lo}")
                pe_w = width - DVE_W
                npe = (pe_w + 511) // 512
                pe_ps = [
                    ps.tile(
                        [1, min(512, pe_w - 512 * i)], dt, tag="b", name=f"pe{lo}_{i}"
                    )
                    for i in range(npe)
                ]
                for k in range(16):
                    lt = stile("lm", 3584)
                    nc.sync.dma_start(
                        lt[:, 0:width], lm_r[:, k * VPC + lo : k * VPC + hi]
                    )
                    if k == 0:
                        nc.vector.tensor_scalar_mul(
                            acc_lm[:], lt[:, 0:DVE_W], x5[:, 0:1]
                        )
                    else:
                        nc.vector.scalar_tensor_tensor(
                            acc_lm[:], lt[:, 0:DVE_W], x5[:, k : k + 1], acc_lm[:],
                            op0=MUL, op1=ADD,
                        )
                    for i in range(npe):
                        cw = pe_ps[i].shape[1]
                        nc.tensor.matmul(
                            pe_ps[i][:], x5[:, k : k + 1],
                            lt[:, DVE_W + i * 512 : DVE_W + i * 512 + cw],
                            start=(k == 0), stop=(k == 15),
                        )
                for n in range(3):
                    pl = ps.tile([1, 512], dt, tag="b", name=f"pla{lo}_{n}")
                    nc.tensor.matmul(
                        pl[:], ones_col[:], acc_lm[:, n * 512 : (n + 1) * 512],
                        start=True, stop=True,
                    )
                    lr = small.tile([1, 512], dt, tag="lrow", name=f"lr{lo}_{n}", bufs=3)
                    nc.vector.tensor_copy(lr[:], pl[:])
                    nc.scalar.dma_start(
                        logits_out[:, lo + n * 512 : lo + (n + 1) * 512], lr[:]
                    )
                for i in range(npe):
                    cw = pe_ps[i].shape[1]
                    lr = small.tile(
                        [1, 512], dt, tag="lrow", name=f"lrp{lo}_{i}", bufs=3
                    )
                    nc.vector.tensor_copy(lr[:, 0:cw], pe_ps[i][:])
                    nc.scalar.dma_start(
                        logits_out[:, lo + DVE_W + i * 512 : lo + DVE_W + i * 512 + cw],
                        lr[:, 0:cw],
                    )

    nc.finalize()
    return nc


def _col16(v):
    """[2048] vector -> [128, 16] column-major layout (e = c*128 + p at [p, c])."""
    return np.ascontiguousarray(v.reshape(16, 128).T)


def _part_major(mT, nblk, blk_rows, width):
    """[nblk*blk_rows, width] -> [blk_rows, nblk*width] partition-major."""
    return np.ascontiguousarray(
        mT.reshape(nblk, blk_rows, width).transpose(1, 0, 2).reshape(blk_rows, nblk * width)
    )


def _prep_in_maps(token_id, pos_id, keys, values, wte, wpe, wq, wk, wv, wo, w1, w2, lm_w):
    f32 = lambda a: np.asarray(a, dtype=np.float32)
    keys, values = f32(keys), f32(values)
    wq, wk, wv, wo, w1, w2, lm_w = map(f32, (wq, wk, wv, wo, w1, w2, lm_w))
    xe_wte = _col16(f32(wte[token_id]))
    xe_wpe = _col16(f32(wpe[pos_id]))
    lm_pad = np.zeros((N_CORES * VPC, E), np.float32)
    lm_pad[:VOCAB] = lm_w

    in_maps = []
    for i in range(N_CORES):
        hs = slice(i * EPC, (i + 1) * EPC)
        wqkv = np.concatenate([wq[hs], wk[hs], wv[hs]], axis=0)  # [768, E]
        in_maps.append(
            {
                "xe_wte": xe_wte,
                "xe_wpe": xe_wpe,
                "wqkv_r": _part_major(np.ascontiguousarray(wqkv.T), 16, 128, 768),
                "keys_r": _part_major(np.ascontiguousarray(keys[:, hs].T), 2, 128, 8192),
                "vals_r": _part_major(values[:, hs], 64, 128, EPC),
                "wo_r": _part_major(np.ascontiguousarray(wo[:, hs].T), 2, 128, E),
                "w1_r": _part_major(
                    np.ascontiguousarray(w1[i * 1024 : (i + 1) * 1024].T), 16, 128, 1024
                ),
                "w2_r": _part_major(
                    np.ascontiguousarray(w2[:, i * 1024 : (i + 1) * 1024].T), 8, 128, E
                ),
                "lm_r": _part_major(
                    np.ascontiguousarray(lm_pad[i * VPC : (i + 1) * VPC].T), 16, 128, VPC
                ),
            }
        )
    return in_maps


def kernel(**inputs) -> np.ndarray:
    from concourse.bass_utils import run_bass_kernel_spmd

    token_id = int(inputs["token_id"])
    pos_id = int(inputs["pos_id"])
    in_maps = _prep_in_maps(
        token_id,
        pos_id,
        inputs["keys"],
        inputs["values"],
        inputs["wte"],
        inputs["wpe"],
        inputs["wq"],
        inputs["wk"],
        inputs["wv"],
        inputs["wo"],
        inputs["w1"],
        inputs["w2"],
        inputs["lm_w"],
    )
    if "nc" not in _CACHE:
        _CACHE["nc"] = _build_nc()
    nc = _CACHE["nc"]
    res = run_bass_kernel_spmd(
        nc,
        in_maps,
        core_ids=list(range(N_CORES)),
        trace=TRACE,
        trace_cores=[0] if TRACE else None,
    )
    _CACHE["last_result"] = res
    logits = np.concatenate([r["logits"][0] for r in res.results])[:VOCAB]
    return np.ascontiguousarray(logits.astype(np.float32))


# revision 15
# speedup vs baseline: 1.0835x; 1.0835x over previous
"""Tensor-parallel MiniGPT single-token decode step on 8 Trainium2 NeuronCores.

Sharding (per core i of 8):
  - attention: heads 2i, 2i+1 (head_dim 128 -> cols i*256:(i+1)*256 of E=2048);
    wq/wk/wv row-sharded, wo column-sharded, KV cache column-sharded by head.
  - MLP: w1 row-sharded (1024 rows/core), w2 column-sharded.
  - LM head: vocab-sharded (50257 padded to 8*6283=50264 rows).
  - Two 8KB AllReduces combine the wo- and w2- partial sums; logits are
    gathered on the host.

All weights are pre-laid-out on the host into [128, F] partition-major arrays
so every device DMA is one contiguous run per partition. All compute is fp32.

Matvec strategy: fp32 PE matmuls with a [128, 1] stationary run at only
~32 MACs/cycle, so the contraction is spread across three engines:
  - DVE / GPSIMD: acc[p, n] += W_tile[p, n] * x[k*128+p] via
    scalar_tensor_tensor, then one PE ones-vector matmul per 512-col chunk
    reduces across partitions (fp32 throughout).
  - PE: direct fp32 matmul chains (lhsT = x chunk) for a share of columns.
This keeps every engine's busy time below the DMA streaming time.
"""

import numpy as np

N_CORES = 8
E = 2048
HPC = 2  # heads per core
EPC = HPC * 128  # 256
T = 8192
VOCAB = 50257
VPC = 6283  # padded vocab rows per core (8 * 6283 = 50264)
SCALE = float(1.0 / np.sqrt(128.0))
EPS = 1e-5

_CACHE = {}
TRACE = False


def _build_nc():
    import concourse.bacc as bacc
    import concourse.mybir as mybir
    import concourse.tile as tile
    from concourse.masks import make_identity

    AF = mybir.ActivationFunctionType
    AX = mybir.AxisListType
    MUL = mybir.AluOpType.mult
    ADD = mybir.AluOpType.add
    dt = mybir.dt.float32

    nc = bacc.Bacc(
        "TRN2", target_bir_lowering=False, debug=False, num_devices=N_CORES
    )

    xe_wte = nc.declare_dram_parameter("xe_wte", [128, 16], dt, isOutput=False)
    xe_wpe = nc.declare_dram_parameter("xe_wpe", [128, 16], dt, isOutput=False)
    wqkv_r = nc.declare_dram_parameter("wqkv_r", [128, 16 * 768], dt, isOutput=False)
    keys_r = nc.declare_dram_parameter("keys_r", [128, 2 * 8192], dt, isOutput=False)
    vals_r = nc.declare_dram_parameter("vals_r", [128, 64 * 256], dt, isOutput=False)
    wo_r = nc.declare_dram_parameter("wo_r", [128, 2 * 2048], dt, isOutput=False)
    w1_r = nc.declare_dram_parameter("w1_r", [128, 16 * 1024], dt, isOutput=False)
    w2_r = nc.declare_dram_parameter("w2_r", [128, 8 * 2048], dt, isOutput=False)
    lm_r = nc.declare_dram_parameter("lm_r", [128, 16 * VPC], dt, isOutput=False)
    logits_out = nc.declare_dram_parameter("logits", [1, VPC], dt, isOutput=True)

    with tile.TileContext(nc) as tc:
        with (
            tc.tile_pool(name="const", bufs=1) as const,
            tc.tile_pool(name="small", bufs=1) as small,
            tc.tile_pool(name="stage", bufs=2) as stage,
            tc.tile_pool(name="ps", bufs=8, space="PSUM") as ps,
            tc.tile_pool(name="dram", bufs=1, space="DRAM") as dram,
            tc.tile_pool(name="stream", bufs=13) as stream,
            tc.tile_pool(name="wqkv", bufs=3) as wqkv_pool,
            tc.tile_pool(name="acc", bufs=2) as accp,
        ):
            _snum = [0]

            def stile(label, width=2048):
                _snum[0] += 1
                return stream.tile(
                    [128, width], dt, tag="s", name=f"s{_snum[0]}_{label}"
                )

            ones_col = const.tile([128, 1], dt)
            nc.vector.memset(ones_col[:], 1.0)
            ones_row = const.tile([1, 128], dt)
            nc.vector.memset(ones_row[:], 1.0)
            ident = const.tile([128, 128], dt)
            make_identity(nc, ident[:])
            eps_c = const.tile([1, 1], dt)
            nc.vector.memset(eps_c[:], EPS)

            # Warm up the collectives path off the critical path: the first
            # collective in a NEFF pays a ~13us init cost.
            warm_in = dram.tile([1, 16], dt, tag="warm_in")
            warm_out = dram.tile([N_CORES, 16], dt, tag="warm_out")
            warm_sb = stage.tile([1, 16], dt, tag="warm", bufs=1)
            nc.vector.memset(warm_sb[:], 0.0)
            nc.scalar.dma_start(warm_in[:], warm_sb[:])
            nc.gpsimd.collective_compute(
                "AllGather",
                mybir.AluOpType.bypass,
                replica_groups=[list(range(N_CORES))],
                ins=[warm_in.opt()],
                outs=[warm_out.opt()],
            )
            warm_back = stage.tile([1, 16], dt, tag="warmb", bufs=1)
            nc.scalar.dma_start(warm_back[:], warm_out[0:1, :])

            def rms(xt, name, extra=None):
                """x * rsqrt(mean(x^2) + eps) for x in [128, 16] column layout.

                extra: optional [1, 16]-shaped nuisance row added with weight 0
                (keeps the warmup collective's output alive against DCE).
                """
                sq = small.tile([128, 16], dt, tag=f"sq_{name}")
                ssum = small.tile([128, 1], dt, tag=f"ss_{name}")
                nc.scalar.activation(sq[:], xt[:], AF.Square, accum_out=ssum[:])
                tot = ps.tile([1, 1], dt, tag="b")
                nc.tensor.matmul(tot[:], ssum[:], ones_col[:], start=True, stop=True)
                std = small.tile([1, 1], dt, tag=f"std_{name}")
                nc.scalar.activation(
                    std[:], tot[:], AF.Sqrt, bias=eps_c[:], scale=1.0 / float(E)
                )
                inv = small.tile([1, 1], dt, tag=f"inv_{name}")
                nc.vector.reciprocal(inv[:], std[:])
                invb_ps = ps.tile([128, 1], dt, tag="b")
                nc.tensor.matmul(
                    invb_ps[:], ones_row[:], inv[:], start=True, stop=True
                )
                invb = small.tile([128, 1], dt, tag=f"invb_{name}")
                nc.vector.tensor_copy(invb[:], invb_ps[:])
                xn = small.tile([128, 16], dt, tag=f"xn_{name}")
                nc.vector.tensor_scalar_mul(xn[:], xt[:], invb[:])
                return xn

            # ---- embedding + double rms ----
            xw = stage.tile([128, 16], dt, tag="xw")
            nc.scalar.dma_start(xw[:], xe_wte[:])
            xp = stage.tile([128, 16], dt, tag="xp")
            nc.scalar.dma_start(xp[:], xe_wpe[:])
            x0 = small.tile([128, 16], dt, tag="x0")
            nc.vector.tensor_add(x0[:], xw[:], xp[:])
            # keep the warmup-collective result alive: x0[0, :] += 0 * warm
            nc.vector.scalar_tensor_tensor(
                x0[0:1, :], warm_back[:], 0.0, x0[0:1, :], op0=MUL, op1=ADD
            )
            x1 = rms(x0, "n1")  # residual input
            x2 = rms(x1, "n2")

            # ---- qkv projection: [1, 768] row (q 0:256 | k 256:512 | v 512:768)
            acc_qkv = accp.tile([128, 768], dt, tag="acc", name="acc_qkv")
            for k in range(16):
                wt = wqkv_pool.tile([128, 768], dt)
                nc.sync.dma_start(wt[:], wqkv_r[:, k * 768 : (k + 1) * 768])
                if k == 0:
                    nc.vector.tensor_scalar_mul(acc_qkv[:], wt[:], x2[:, 0:1])
                else:
                    nc.vector.scalar_tensor_tensor(
                        acc_qkv[:], wt[:], x2[:, k : k + 1], acc_qkv[:],
                        op0=MUL, op1=ADD,
                    )
            ps_q = ps.tile([1, 512], dt, tag="b")
            nc.tensor.matmul(ps_q[:], ones_col[:], acc_qkv[:, 0:512], start=True, stop=True)
            ps_v = ps.tile([1, 256], dt, tag="b")
            nc.tensor.matmul(ps_v[:], ones_col[:], acc_qkv[:, 512:768], start=True, stop=True)
            qkv_row = small.tile([1, 768], dt, tag="qkv")
            nc.scalar.mul(qkv_row[:, 0:256], ps_q[:, 0:256], SCALE)
            nc.scalar.copy(qkv_row[:, 256:512], ps_q[:, 256:512])
            nc.scalar.copy(qkv_row[:, 512:768], ps_v[:])

            # ---- transpose q,k to column layout: qkT[:, 0:2]=q heads, 2:4=k heads
            st4 = stage.tile([4, 128], dt, tag="st4")
            nc.scalar.dma_start(st4[:], qkv_row[:, 0:512])
            qkT_ps = ps.tile([128, 4], dt, tag="b")
            nc.tensor.transpose(qkT_ps[:], st4[:], ident[0:4, 0:4])
            qkT = small.tile([128, 4], dt, tag="qkT")
            nc.vector.tensor_copy(qkT[:], qkT_ps[:])

            # ---- attention scores, computed directly transposed: for each
            # 128-score chunk c, matmul(lhsT=keysT chunk [128d, 128t],
            # rhs=q [128d, 1]) writes att^T into a PSUM column; exp lands in
            # wT_h[p, c] = exp(att_h[c*128 + p]) (unnormalized).
            wTs = []
            esp = small.tile([128, 16], dt, tag="esp")  # per-partition exp sums
            for h in range(HPC):
                wTs.append(small.tile([128, 64], dt, tag=f"wT{h}", name=f"wT{h}"))
            for j in range(4):
                for h in range(HPC):
                    kt = stile("key")
                    nc.sync.dma_start(
                        kt[:],
                        keys_r[:, h * 8192 + j * 2048 : h * 8192 + (j + 1) * 2048],
                    )
                    wps = ps.tile([128, 16], dt, tag="b", name=f"wps{h}_{j}")
                    for c in range(16):
                        nc.tensor.matmul(
                            wps[:, c : c + 1],
                            kt[:, c * 128 : (c + 1) * 128],
                            qkT[:, h : h + 1],
                            start=True, stop=True,
                        )
                    nc.scalar.activation(
                        wTs[h][:, j * 16 : (j + 1) * 16], wps[:], AF.Exp,
                        accum_out=esp[:, h * 4 + j : h * 4 + j + 1],
                    )

            # current-token score per head: exp(q_h . k_h) (SCALE folded into q)
            e_last = small.tile([1, 2], dt, tag="elast")
            for h in range(HPC):
                pal = ps.tile([1, 1], dt, tag="b")
                nc.tensor.matmul(
                    pal[:], qkT[:, h : h + 1], qkT[:, 2 + h : 3 + h],
                    start=True, stop=True,
                )
                nc.scalar.activation(e_last[:, h : h + 1], pal[:], AF.Exp)

            # softmax denominators: cross-partition sum of esp + e_last
            dps = ps.tile([1, 8], dt, tag="b")
            nc.tensor.matmul(dps[:], ones_col[:], esp[:, 0:8], start=True, stop=True)
            dtmp = small.tile([1, 2], dt, tag="dtmp")
            for h in range(HPC):
                nc.vector.reduce_sum(
                    dtmp[:, h : h + 1], dps[:, h * 4 : (h + 1) * 4], axis=AX.X
                )
            nc.vector.tensor_add(dtmp[:], dtmp[:], e_last[:])
            dinv = small.tile([1, 2], dt, tag="dinv")
            nc.vector.reciprocal(dinv[:], dtmp[:])

            # ---- PV: x_attn_h = sum_t w[t] * V[t, :] (unnormalized) on DVE
            acc_pv = [
                accp.tile([128, 128], dt, tag=f"accpv{h}", name=f"acc_pv{h}")
                for h in range(HPC)
            ]
            for tt in range(8):
                vt = stile("val")
                nc.sync.dma_start(vt[:], vals_r[:, tt * 2048 : (tt + 1) * 2048])
                for j in range(8):
                    c = tt * 8 + j
                    for h in range(HPC):
                        vsl = vt[:, j * 256 + h * 128 : j * 256 + (h + 1) * 128]
                        if c == 0:
                            nc.vector.tensor_scalar_mul(
                                acc_pv[h][:], vsl, wTs[h][:, 0:1]
                            )
                        else:
                            nc.vector.scalar_tensor_tensor(
                                acc_pv[h][:], vsl, wTs[h][:, c : c + 1], acc_pv[h][:],
                                op0=MUL, op1=ADD,
                            )
            pv_ps = []
            for h in range(HPC):
                p = ps.tile([1, 128], dt, tag="b", name=f"pv_ps{h}")
                nc.tensor.matmul(p[:], ones_col[:], acc_pv[h][:], start=True, stop=True)
                pv_ps.append(p)

            # combine with current-token value, then normalize by the softmax sum
            xa_row = small.tile([1, 256], dt, tag="xa")
            for h in range(HPC):
                sl = slice(h * 128, (h + 1) * 128)
                nc.vector.tensor_scalar_mul(
                    xa_row[:, sl],
                    qkv_row[:, 512 + h * 128 : 512 + (h + 1) * 128],
                    e_last[:, h : h + 1],
                )
                nc.vector.tensor_add(xa_row[:, sl], xa_row[:, sl], pv_ps[h][:])
                nc.vector.tensor_scalar_mul(xa_row[:, sl], xa_row[:, sl], dinv[:, h : h + 1])

            # ---- transpose x_attn to column layout [128, 2] ----
            st2 = stage.tile([2, 128], dt, tag="st2")
            nc.scalar.dma_start(st2[:], xa_row[:])
            xaT_ps = ps.tile([128, 2], dt, tag="b")
            nc.tensor.transpose(xaT_ps[:], st2[:], ident[0:2, 0:2])
            xaT = small.tile([128, 2], dt, tag="xaT")
            nc.vector.tensor_copy(xaT[:], xaT_ps[:])

            # ---- wo partial: [1, 2048]; DVE cols 0:1024, PE chains 1024:2048
            ar1_in = small.tile([1, 2048], dt, tag="arin", name="ar1_in")
            acc_wo = accp.tile([128, 1024], dt, tag="acc", name="acc_wo")
            wo_pe = [ps.tile([1, 512], dt, tag="b", name=f"wope{n}") for n in range(2)]
            for k in range(2):
                wot = stile("wo")
                nc.sync.dma_start(wot[:], wo_r[:, k * 2048 : (k + 1) * 2048])
                if k == 0:
                    nc.vector.tensor_scalar_mul(acc_wo[:], wot[:, 0:1024], xaT[:, 0:1])
                else:
                    nc.vector.scalar_tensor_tensor(
                        acc_wo[:], wot[:, 0:1024], xaT[:, 1:2], acc_wo[:],
                        op0=MUL, op1=ADD,
                    )
                for n in range(2):
                    nc.tensor.matmul(
                        wo_pe[n][:], xaT[:, k : k + 1],
                        wot[:, 1024 + n * 512 : 1024 + (n + 1) * 512],
                        start=(k == 0), stop=(k == 1),
                    )
            for n in range(2):
                po = ps.tile([1, 512], dt, tag="b")
                nc.tensor.matmul(
                    po[:], ones_col[:], acc_wo[:, n * 512 : (n + 1) * 512],
                    start=True, stop=True,
                )
                nc.vector.tensor_copy(ar1_in[:, n * 512 : (n + 1) * 512], po[:])
            for n in range(2):
                nc.vector.tensor_copy(
                    ar1_in[:, 1024 + n * 512 : 1024 + (n + 1) * 512], wo_pe[n][:]
                )

            def all_reduce(row_sb, name):
                """Sum a [1, 2048] partial across cores; returns PSUM [128, 16].

                AllGather + on-core rank reduction: 16 K=8 matmuls against a
                ones vector sum the 8 gathered rows and transpose into the
                [128, 16] column layout.
                """
                in_d = dram.tile([1, 2048], dt, tag=f"{name}_in")
                out_d = dram.tile([N_CORES, 2048], dt, tag=f"{name}_out")
                nc.scalar.dma_start(in_d[:], row_sb[:])
                nc.gpsimd.collective_compute(
                    "AllGather",
                    mybir.AluOpType.bypass,
                    replica_groups=[list(range(N_CORES))],
                    ins=[in_d.opt()],
                    outs=[out_d.opt()],
                )
                ag_sb = stage.tile(
                    [N_CORES, 2048], dt, tag="ag", name=f"ag_{name}", bufs=1
                )
                nc.scalar.dma_start(ag_sb[:], out_d[:])
                x_ps = ps.tile([128, 16], dt, tag="b", name=f"xps_{name}")
                for c in range(16):
                    nc.tensor.matmul(
                        x_ps[:, c : c + 1],
                        ag_sb[:, c * 128 : (c + 1) * 128],
                        ones_col[0:N_CORES, :],
                        start=True, stop=True,
                    )
                return x_ps

            x3_ps = all_reduce(ar1_in, "ar1")
            x3 = small.tile([128, 16], dt, tag="x3")
            nc.vector.tensor_add(x3[:], x3_ps[:], x1[:])  # + residual

            # ---- MLP1: h = relu(w1 @ x4) on DVE ----
            x4 = rms(x3, "n3")
            acc_h1 = accp.tile([128, 1024], dt, tag="acc", name="acc_h1")
            for a in range(8):
                w1t = stile("w1")
                nc.sync.dma_start(w1t[:], w1_r[:, a * 2048 : (a + 1) * 2048])
                for b in range(2):
                    k = a * 2 + b
                    wsl = w1t[:, b * 1024 : (b + 1) * 1024]
                    if k == 0:
                        nc.vector.tensor_scalar_mul(acc_h1[:], wsl, x4[:, 0:1])
                    else:
                        nc.vector.scalar_tensor_tensor(
                            acc_h1[:], wsl, x4[:, k : k + 1], acc_h1[:],
                            op0=MUL, op1=ADD,
                        )
            h_row = small.tile([1, 1024], dt, tag="hrow")
            for n in range(2):
                phn = ps.tile([1, 512], dt, tag="b", name=f"ph{n}")
                nc.tensor.matmul(
                    phn[:], ones_col[:], acc_h1[:, n * 512 : (n + 1) * 512],
                    start=True, stop=True,
                )
                nc.scalar.activation(h_row[:, n * 512 : (n + 1) * 512], phn[:], AF.Relu)

            st8 = stage.tile([8, 128], dt, tag="st8")
            nc.scalar.dma_start(st8[:], h_row[:])
            hT_ps = ps.tile([128, 8], dt, tag="b")
            nc.tensor.transpose(hT_ps[:], st8[:], ident[0:8, 0:8])
            hT = small.tile([128, 8], dt, tag="hT")
            nc.vector.tensor_copy(hT[:], hT_ps[:])

            # ---- MLP2: DVE cols 0:1024 of each k-block, PE chains 1024:2048
            ar2_in = small.tile([1, 2048], dt, tag="arin", name="ar2_in")
            acc_m2 = accp.tile([128, 1024], dt, tag="acc", name="acc_m2")
            pm_pe = [ps.tile([1, 512], dt, tag="b", name=f"pmpe{n}") for n in range(2)]
            for k in range(8):
                w2t = stile("w2")
                nc.sync.dma_start(w2t[:], w2_r[:, k * 2048 : (k + 1) * 2048])
                if k == 0:
                    nc.vector.tensor_scalar_mul(acc_m2[:], w2t[:, 0:1024], hT[:, 0:1])
                else:
                    nc.vector.scalar_tensor_tensor(
                        acc_m2[:], w2t[:, 0:1024], hT[:, k : k + 1], acc_m2[:],
                        op0=MUL, op1=ADD,
                    )
                for n in range(2):
                    nc.tensor.matmul(
                        pm_pe[n][:], hT[:, k : k + 1],
                        w2t[:, 1024 + n * 512 : 1024 + (n + 1) * 512],
                        start=(k == 0), stop=(k == 7),
                    )
            for n in range(2):
                pm = ps.tile([1, 512], dt, tag="b", name=f"pm{n}")
                nc.tensor.matmul(
                    pm[:], ones_col[:], acc_m2[:, n * 512 : (n + 1) * 512],
                    start=True, stop=True,
                )
                nc.vector.tensor_copy(ar2_in[:, n * 512 : (n + 1) * 512], pm[:])
            for n in range(2):
                nc.vector.tensor_copy(
                    ar2_in[:, 1024 + n * 512 : 1024 + (n + 1) * 512], pm_pe[n][:]
                )

            x5_ps = all_reduce(ar2_in, "ar2")
            x5 = small.tile([128, 16], dt, tag="x5")
            nc.vector.tensor_add(x5[:], x5_ps[:], x3[:])  # + residual (x3)

            # ---- LM head over the vocab shard, two passes of 512-col chunks.
            # Per pass/k: lt_a [128, 2048] + lt_b [128, w-2048]; DVE accumulates
            # lt_a cols 0:1536, PE runs direct chains for the remaining columns.
            passes = [(0, 3584), (3584, VPC)]
            DVE_W = 1536
            for lo, hi in passes:
                width = hi - lo
                w_b = width - 2048
                acc_lm = accp.tile([128, DVE_W], dt, tag="acclm", name=f"acc_lm{lo}")
                pe_w = width - DVE_W
                npe = (pe_w + 511) // 512
                pe_ps = [
                    ps.tile(
                        [1, min(512, pe_w - 512 * i)], dt, tag="b", name=f"pe{lo}_{i}"
                    )
                    for i in range(npe)
                ]
                for k in range(16):
                    lt_a = stile("lma")
                    nc.sync.dma_start(
                        lt_a[:], lm_r[:, k * VPC + lo : k * VPC + lo + 2048]
                    )
                    lt_b = stile("lmb")
                    nc.sync.dma_start(
                        lt_b[:, 0:w_b], lm_r[:, k * VPC + lo + 2048 : k * VPC + hi]
                    )
                    if k == 0:
                        nc.vector.tensor_scalar_mul(
                            acc_lm[:], lt_a[:, 0:DVE_W], x5[:, 0:1]
                        )
                    else:
                        nc.vector.scalar_tensor_tensor(
                            acc_lm[:], lt_a[:, 0:DVE_W], x5[:, k : k + 1], acc_lm[:],
                            op0=MUL, op1=ADD,
                        )
                    for i in range(npe):
                        cw = pe_ps[i].shape[1]
                        coff = DVE_W + i * 512
                        if coff + cw <= 2048:
                            rhs = lt_a[:, coff : coff + cw]
                        else:
                            rhs = lt_b[:, coff - 2048 : coff - 2048 + cw]
                        nc.tensor.matmul(
                            pe_ps[i][:], x5[:, k : k + 1], rhs,
                            start=(k == 0), stop=(k == 15),
                        )
                for n in range(3):
                    pl = ps.tile([1, 512], dt, tag="b", name=f"pla{lo}_{n}")
                    nc.tensor.matmul(
                        pl[:], ones_col[:], acc_lm[:, n * 512 : (n + 1) * 512],
                        start=True, stop=True,
                    )
                    lr = small.tile([1, 512], dt, tag="lrow", name=f"lr{lo}_{n}", bufs=3)
                    nc.vector.tensor_copy(lr[:], pl[:])
                    nc.scalar.dma_start(
                        logits_out[:, lo + n * 512 : lo + (n + 1) * 512], lr[:]
                    )
                for i in range(npe):
                    cw = pe_ps[i].shape[1]
                    lr = small.tile(
                        [1, 512], dt, tag="lrow", name=f"lrp{lo}_{i}", bufs=3
                    )
                    nc.vector.tensor_copy(lr[:, 0:cw], pe_ps[i][:])
                    nc.scalar.dma_start(
                        logits_out[:, lo + DVE_W + i * 512 : lo + DVE_W + i * 512 + cw],
                        lr[:, 0:cw],
                    )

    nc.finalize()
    return nc


def _col16(v):
    """[2048] vector -> [128, 16] column-major layout (e = c*128 + p at [p, c])."""
    return np.ascontiguousarray(v.reshape(16, 128).T)


def _part_major(mT, nblk, blk_rows, width):
    """[nblk*blk_rows, width] -> [blk_rows, nblk*width] partition-major."""
    return np.ascontiguousarray(
        mT.reshape(nblk, blk_rows, width).transpose(1, 0, 2).reshape(blk_rows, nblk * width)
    )


def _prep_in_maps(token_id, pos_id, keys, values, wte, wpe, wq, wk, wv, wo, w1, w2, lm_w):
    f32 = lambda a: np.asarray(a, dtype=np.float32)
    keys, values = f32(keys), f32(values)
    wq, wk, wv, wo, w1, w2, lm_w = map(f32, (wq, wk, wv, wo, w1, w2, lm_w))
    xe_wte = _col16(f32(wte[token_id]))
    xe_wpe = _col16(f32(wpe[pos_id]))
    lm_pad = np.zeros((N_CORES * VPC, E), np.float32)
    lm_pad[:VOCAB] = lm_w

    in_maps = []
    for i in range(N_CORES):
        hs = slice(i * EPC, (i + 1) * EPC)
        wqkv = np.concatenate([wq[hs], wk[hs], wv[hs]], axis=0)  # [768, E]
        in_maps.append(
            {
                "xe_wte": xe_wte,
                "xe_wpe": xe_wpe,
                "wqkv_r": _part_major(np.ascontiguousarray(wqkv.T), 16, 128, 768),
                "keys_r": _part_major(np.ascontiguousarray(keys[:, hs].T), 2, 128, 8192),
                "vals_r": _part_major(values[:, hs], 64, 128, EPC),
                "wo_r": _part_major(np.ascontiguousarray(wo[:, hs].T), 2, 128, E),
                "w1_r": _part_major(
                    np.ascontiguousarray(w1[i * 1024 : (i + 1) * 1024].T), 16, 128, 1024
                ),
                "w2_r": _part_major(
                    np.ascontiguousarray(w2[:, i * 1024 : (i + 1) * 1024].T), 8, 128, E
                ),
                "lm_r": _part_major(
                    np.ascontiguousarray(lm_pad[i * VPC : (i + 1) * VPC].T), 16, 128, VPC
                ),
            }
        )
    return in_maps


def kernel(**inputs) -> np.ndarray:
    from concourse.bass_utils import run_bass_kernel_spmd

    token_id = int(inputs["token_id"])
    pos_id = int(inputs["pos_id"])
    in_maps = _prep_in_maps(
        token_id,
        pos_id,
        inputs["keys"],
        inputs["values"],
        inputs["wte"],
        inputs["wpe"],
        inputs["wq"],
        inputs["wk"],
        inputs["wv"],
        inputs["wo"],
        inputs["w1"],
        inputs["w2"],
        inputs["lm_w"],
    )
    if "nc" not in _CACHE:
        _CACHE["nc"] = _build_nc()
    nc = _CACHE["nc"]
    res = run_bass_kernel_spmd(
        nc,
        in_maps,
        core_ids=list(range(N_CORES)),
        trace=TRACE,
        trace_cores=[0] if TRACE else None,
    )
    _CACHE["last_result"] = res
    logits = np.concatenate([r["logits"][0] for r in res.results])[:VOCAB]
    return np.ascontiguousarray(logits.astype(np.float32))
